# revision 10
# baseline (speedup 1.0000x reference)
"""Multi-head attention kernel for Trainium2, distributed over 8 NeuronCores.

Problem: x[8,1,2048,384] @ W_qkv[384,1152] -> 8-head attention (dk=48,
softmax scale 1/sqrt(2048)) -> @ W_o[384,384] + b_o.

Sharding: batch (b=8) data-parallel, one batch element per core. No
collectives.

The wall-clock floor is ScalarE: softmax needs exp of h*n^2 = 33.5M
elements/core, and ScalarE (the only exp engine; GPSIMD has no PSUM
port, DVE has no exp) streams 128 lanes @ 1.2 GHz. The kernel's whole
job is to keep the 256 [128,1024] exp ops back-to-back forever:

  1. Per n-tile prep: xT via PE transpose (fp32 exact, stored bf16); v
     projection packed as v_pack[t][128, h, 64] = [v48 | ones16] so the PV
     matmul also emits the softmax denominator Z (rows 48:64 of each
     head's output strip). q/k projections computed TRANSPOSED ([dk, n])
     in bf16, two heads per tile (rows 0:48 / 64:112, col-packed via
     tile_position); PSUM->SBUF copies on VectorE so ScalarE's FIFO only
     ever holds exps.
  2. Attention, pair-outer: per (pair, c5 chunk of 512 q, t of 16
     k-tiles): the two heads' S^T matmuls write ONE 2-bank PSUM tile and
     run CONCURRENTLY on disjoint PE row strips; one [128, 1024] exp per
     tile straight from PSUM (scale folded in), P^T bf16. PV lags one
     tile: both heads' PV matmuls write ONE 1-bank PSUM tile oAB (head A
     rows 0:64, head B rows 64:128 via tile_position).
  3. All prep work (x DMA, transposes, v/q/k projections) and the fc_o
     chunks are deadline-scheduled one small group per attention window,
     CROSS-REP software pipelined: rep r's prep runs inside rep r-1's
     windows (xT and v_pack are rep-parity double-buffered so there are
     no WAR hazards), so at a rep boundary the PE's in-order queue goes
     straight from rep r-1's last S tile to rep r's first one and the
     exp stream never pauses.
  4. Normalization: DVE reciprocal of the 16 Z rows (over a 32-aligned
     partition superset), partition-replicate x3 via DMA, DVE multiply ->
     dense attn^T [384, n] f32r via repack DMA.
  5. fc_o: 3 matmuls per n-tile at full K=128 (PSUM tiles borrowed from
     the prep pool), bias added on DVE against a DMA-broadcast b_o;
     chunk j is scheduled into the windows right after pair 3 finishes
     chunk j, the last chunk into the next rep's early windows.

Measured on TRN2 (rep-differenced): see test log. Max rel err ~2.5e-3 vs
fp32 reference (bf16 operand rounding).
"""

import numpy as np

import concourse.bass as bass
import concourse.mybir as mybir
import concourse.tile as tile
from concourse import bacc
from concourse.bass_utils import run_bass_kernel_spmd
from concourse.masks import make_identity

F32 = mybir.dt.float32
F32R = mybir.dt.float32r
BF16 = mybir.dt.bfloat16
AF = mybir.ActivationFunctionType

N = 2048          # sequence length per core
D = 384           # d_model
H = 8             # heads
DK = 48           # head dim
NCORES = 8
SCALE = 1.0 / float(np.sqrt(N))  # reference scales by sqrt(seq), not sqrt(dk)

NT = N // 128     # 16 n-tiles of 128
NC_Q = 2          # n_q chunks for attn_dense
CQ = N // NC_Q    # 1024-wide n_q chunks
DT3 = D // 128    # 3 d-model chunks
WPR = 256         # attention windows per rep

QK_DT = BF16
PT_DT = BF16
V_DT = BF16


def build_nc(reps=1, stages="absepnf"):
    # stages: a=xT+vproj, b=qkproj, s=S-matmuls, e=exp, p=PV, n=norm+repack, f=fc_o
    nc = bacc.Bacc(debug=False)
    x = nc.declare_dram_parameter("x", [N, D], F32, isOutput=False).ap()
    w_qkv = nc.declare_dram_parameter("W_qkv", [D, 3 * D], F32, isOutput=False).ap()
    w_o = nc.declare_dram_parameter("W_o", [D, D], F32, isOutput=False).ap()
    b_o = nc.declare_dram_parameter("b_o", [D], F32, isOutput=False).ap()
    out = nc.declare_dram_parameter("out", [N, D], F32, isOutput=True).ap()

    with tile.TileContext(nc) as tc:
        _emit(nc, tc, x, w_qkv, w_o, b_o, out, reps, stages)
    nc.compile()
    return nc


def _emit(nc, tc, x, w_qkv, w_o, b_o, out, reps=1, stages="absepnf"):
    from contextlib import ExitStack

    ctx = ExitStack()
    with ctx:
        persist = ctx.enter_context(tc.tile_pool(name="persist", bufs=1))

        # --- constants -----------------------------------------------------
        ident = persist.tile([128, 128], F32)
        make_identity(nc, ident)

        # W_qkv as 3 d-chunk tiles [128, 1152] in bf16 (the whole qkv
        # projection runs bf16: f32r + col-packed tile_position fails the
        # walrus ISA check, and q/k/v are stored bf16 downstream anyway).
        wqkv_sb = []
        with tc.tile_pool(name="wstage", bufs=2) as wstage:
            for dc in range(DT3):
                w_stage = wstage.tile([128, 3 * D], F32)
                nc.sync.dma_start(out=w_stage, in_=w_qkv[dc * 128 : (dc + 1) * 128, :])
                w_t = persist.tile([128, 3 * D], BF16, tag=f"wqkv{dc}", name=f"wqkv{dc}")
                nc.vector.tensor_copy(w_t, w_stage)
                wqkv_sb.append(w_t)

        # W_o natural layout, 3 d-chunk tiles [128, 384] f32r
        wo_sb = []
        for dc in range(DT3):
            wo_t = persist.tile([128, D], F32R, tag=f"wo{dc}", name=f"wo{dc}")
            nc.sync.dma_start(
                out=wo_t, in_=w_o[dc * 128 : (dc + 1) * 128, :].bitcast(F32R)
            )
            wo_sb.append(wo_t)

        # b_o broadcast to all 128 partitions
        b_bcast = persist.tile([128, D], F32)
        b_src = bass.AP(tensor=b_o.tensor, offset=0, ap=[[0, 128], [1, D]])
        nc.sync.dma_start(out=b_bcast, in_=b_src)

        # --- persistent arrays ---------------------------------------------
        # xT and v_pack are rep-parity double-buffered so rep r's prep can
        # run inside rep r-1's attention windows with no WAR hazards.
        xT = [
            [
                persist.tile([128, N], BF16, tag=f"xT{pb}_{dc}", name=f"xT{pb}_{dc}")
                for dc in range(DT3)
            ]
            for pb in range(2)
        ]
        v_pack = [
            [
                persist.tile([128, H, 64], V_DT, tag=f"vp{pb}_{nt}", name=f"vp{pb}_{nt}")
                for nt in range(NT)
            ]
            for pb in range(2)
        ]
        q_pack = [
            persist.tile([128, N], QK_DT, tag=f"qp{p}", name=f"qp{p}")
            for p in range(H // 2)
        ]
        k_pack = [
            persist.tile([128, N], QK_DT, tag=f"kp{p}", name=f"kp{p}")
            for p in range(H // 2)
        ]
        # dense attn^T: attn_dense[c][dtile]: [128, CQ] f32r
        attn_dense = [
            [
                persist.tile([128, CQ], F32R, tag=f"ad{c}_{d_}", name=f"ad{c}_{d_}")
                for d_ in range(DT3)
            ]
            for c in range(NC_Q)
        ]

        # --- stream pools (span all reps) ----------------------------------
        xload = ctx.enter_context(tc.tile_pool(name="xload", bufs=8))
        # one shared 1-bank-slot PSUM pool for transposes, v/q/k projection
        # and fc_o accumulators; 2 banks total
        prep = ctx.enter_context(tc.tile_pool(name="prep", bufs=2, space="PSUM"))
        spsum = ctx.enter_context(tc.tile_pool(name="spsum", bufs=2, space="PSUM"))
        opsum = ctx.enter_context(tc.tile_pool(name="opsum", bufs=2, space="PSUM"))
        ptpool = ctx.enter_context(tc.tile_pool(name="ptpool", bufs=4))
        zpool = ctx.enter_context(tc.tile_pool(name="zpool", bufs=2))
        stpool = ctx.enter_context(tc.tile_pool(name="stpool", bufs=2))
        fout = ctx.enter_context(tc.tile_pool(name="fout", bufs=3))

        x_tiles = {}

        def emit_xdma(rep, nt):
            x_t = xload.tile([128, D], F32, tag="xt")
            nc.sync.dma_start(out=x_t, in_=x[nt * 128 : (nt + 1) * 128, :])
            x_tiles[(rep, nt)] = x_t

        def emit_a(rep, nt):
            pb = rep % 2
            x_t = x_tiles.pop((rep, nt))
            for dc in range(DT3):
                p_t = prep.tile([128, 128], F32, tag="prep")
                nc.tensor.transpose(p_t, x_t[:, dc * 128 : (dc + 1) * 128], ident)
                nc.vector.tensor_copy(xT[pb][dc][:, nt * 128 : (nt + 1) * 128], p_t)
            pv = prep.tile([128, D], F32, tag="prep")
            for dc in range(DT3):
                nc.tensor.matmul(
                    pv, xT[pb][dc][:, nt * 128 : (nt + 1) * 128],
                    wqkv_sb[dc][:, 2 * D : 3 * D],
                    start=(dc == 0), stop=(dc == DT3 - 1),
                )
            vp = v_pack[pb][nt]
            nc.gpsimd.memset(vp, 1.0)
            pv_h = pv.rearrange("p (h c) -> p h c", c=DK)
            nc.vector.tensor_copy(vp[:, :, 0:DK], pv_h)

        def emit_proj(rep, pair, qk, c4):
            # one projection group: q or k for one head-pair, one 512-col
            # n-chunk; ~0.64us PE (3 dual-strip matmul slots) + one DVE copy
            pb = rep % 2
            hA, hB = 2 * pair, 2 * pair + 1
            dest = q_pack[pair] if qk == 0 else k_pack[pair]
            qoff = 0 if qk == 0 else D
            cs = slice(c4 * 512, (c4 + 1) * 512)
            pp = prep.tile([128, 512], F32, tag="prep")
            for dc in range(DT3):
                nc.tensor.matmul(
                    pp[0:48, :],
                    wqkv_sb[dc][:, qoff + hA * DK : qoff + hA * DK + DK],
                    xT[pb][dc][:, cs],
                    start=(dc == 0), stop=(dc == DT3 - 1),
                )
                nc.tensor.matmul(
                    pp[64:112, :],
                    wqkv_sb[dc][:, qoff + hB * DK : qoff + hB * DK + DK],
                    xT[pb][dc][:, cs],
                    start=(dc == 0), stop=(dc == DT3 - 1),
                    tile_position=(0, 64),
                )
            nc.vector.tensor_copy(dest[0:112, cs], pp[0:112, :])

        def emit_fc(rep, nt):
            # one fc_o n-tile: out[nt] = attn^T[:, nt].T @ W_o + b_o
            c = (nt * 128) // CQ
            col = (nt * 128) % CQ
            cslice = slice(col, col + 128)
            pf = prep.tile([128, D], F32, tag="prep")
            for dc in range(DT3):
                nc.tensor.matmul(
                    pf,
                    attn_dense[c][dc][:, cslice],
                    wo_sb[dc],
                    start=(dc == 0),
                    stop=(dc == DT3 - 1),
                )
            o_t = fout.tile([128, D], F32, tag="fo")
            nc.vector.tensor_add(o_t, pf, b_bcast)
            nc.sync.dma_start(out=out[nt * 128 : (nt + 1) * 128, :], in_=o_t)

        # --- global window schedule ----------------------------------------
        # window g = rep*WPR + pair*64 + c5*16 + t
        sched = {}

        def put(g, fn, *args):
            sched.setdefault(g, []).append((fn, args))

        prologue = []
        for rep in range(reps):
            g0 = rep * WPR
            if "a" in stages:
                if rep == 0:
                    for nt in range(8):
                        prologue.append((emit_xdma, (0, nt)))
                    for nt in range(8):
                        prologue.append((emit_a, (0, nt)))
                    for nt in range(8, NT):
                        put(g0 + (nt - 8), emit_xdma, 0, nt)
                    for nt in range(8, 12):
                        put(g0 + (nt - 8), emit_a, 0, nt)
                    for nt in range(12, NT):
                        put(g0 + (nt - 7), emit_a, 0, nt)
                else:
                    # rep r prep runs inside rep r-1's windows
                    for nt in range(NT):
                        put(g0 - 100 + 4 * nt, emit_xdma, rep, nt)
                        put(g0 - 76 + 4 * nt, emit_a, rep, nt)
            if "b" in stages:
                if rep == 0:
                    prologue.append((emit_proj, (0, 0, 0, 0)))
                    prologue.append((emit_proj, (0, 0, 1, 0)))
                    put(g0 + 0, emit_proj, 0, 0, 1, 1)
                    put(g0 + 4, emit_proj, 0, 0, 1, 2)
                    put(g0 + 9, emit_proj, 0, 0, 1, 3)
                    rest = [(0, c4) for c4 in range(1, 4)]
                else:
                    # pair-0 projections land in rep r-1's last windows
                    put(g0 - 10, emit_proj, rep, 0, 0, 0)
                    put(g0 - 8, emit_proj, rep, 0, 1, 0)
                    put(g0 - 6, emit_proj, rep, 0, 1, 1)
                    put(g0 - 4, emit_proj, rep, 0, 1, 2)
                    put(g0 - 2, emit_proj, rep, 0, 1, 3)
                    rest = [(0, c4) for c4 in range(1, 4)]
                # remaining groups: deadline-scheduled inside this rep
                items = []
                for qk, c4 in rest:
                    items.append((0 * 64 + (c4 * 16 if qk == 0 else 4 * c4), (0, qk, c4)))
                for p in range(1, H // 2):
                    for c4 in range(4):
                        items.append((p * 64 + c4 * 16, (p, 0, c4)))
                        items.append((p * 64 + 4 * c4, (p, 1, c4)))
                items.sort()
                next_free = 12 if rep == 0 else 2
                for deadline, args in items:
                    w = max(next_free, deadline - 12)
                    next_free = w + 1
                    put(g0 + w, emit_proj, rep, *args)
            if "f" in stages:
                # fc chunk j (n-tiles 4j..4j+3) is ready once pair 3
                # finishes c5=j (window g0 + 192 + 16j + 15) plus the
                # norm/repack latency; the last chunk spills into the next
                # rep's early windows (true tail for the last rep).
                for j in range(4):
                    if j < 3:
                        gj = g0 + 192 + 16 * (j + 1) + 2
                    elif rep < reps - 1:
                        gj = g0 + WPR + 6
                    else:
                        gj = reps * WPR  # tail
                    for i in range(4):
                        put(gj + i, emit_fc, rep, 4 * j + i)

        for fn, args in prologue:
            fn(*args)

        # --- attention stream ----------------------------------------------
        pend = None
        oAB = None

        for g in range(reps * WPR):
            for fn, args in sched.pop(g, ()):
                fn(*args)
            if "s" not in stages:
                continue
            rep, w = divmod(g, WPR)
            pair, w2 = divmod(w, 64)
            c5, t = divmod(w2, 16)
            pb = rep % 2
            hA, hB = 2 * pair, 2 * pair + 1
            qp, kp = q_pack[pair], k_pack[pair]
            cqs = slice(c5 * 512, (c5 + 1) * 512)
            if t == 0:
                # both heads' PV accumulate into ONE 1-bank PSUM tile:
                # head A rows 0:64 (48 data + 16 Z), head B rows 64:128.
                oAB = opsum.tile([128, 512], F32, tag="oAB")
                pend = None

            ts_ = slice(t * 128, (t + 1) * 128)
            sAB = spsum.tile([128, 1024], F32, tag="sAB")
            nc.tensor.matmul(
                sAB[:, 0:512], kp[0:48, ts_], qp[0:48, cqs],
                start=True, stop=True,
            )
            nc.tensor.matmul(
                sAB[:, 512:1024], kp[64:112, ts_], qp[64:112, cqs],
                start=True, stop=True,
            )
            if "e" in stages:
                ptAB = ptpool.tile([128, 1024], PT_DT, tag="ptAB")
                nc.scalar.activation(ptAB, sAB, AF.Exp, scale=SCALE)
                if "p" in stages:
                    if pend is not None:
                        tp, pt_prev = pend
                        nc.tensor.matmul(
                            oAB[0:64, :], v_pack[pb][tp][:, hA, :], pt_prev[:, 0:512],
                            start=(tp == 0), stop=False,
                        )
                        nc.tensor.matmul(
                            oAB[64:128, :], v_pack[pb][tp][:, hB, :],
                            pt_prev[:, 512:1024],
                            start=(tp == 0), stop=False,
                            tile_position=(0, 64),
                        )
                    pend = (t, ptAB)

            if t != NT - 1:
                continue
            # --- end of (pair, c5): flush PV, normalize, repack ------------
            if "p" in stages and "e" in stages and pend is not None:
                tp, pt_prev = pend
                nc.tensor.matmul(
                    oAB[0:64, :], v_pack[pb][tp][:, hA, :], pt_prev[:, 0:512],
                    start=(tp == 0), stop=True,
                )
                nc.tensor.matmul(
                    oAB[64:128, :], v_pack[pb][tp][:, hB, :], pt_prev[:, 512:1024],
                    start=(tp == 0), stop=True,
                    tile_position=(0, 64),
                )
                pend = None
            if "n" not in stages:
                continue
            # normalization: Z sits in rows 48:64 (head A) and 112:128
            # (head B) of oAB, replicated over 16 rows by the 16 ones-
            # columns.  Engine partition access must be 32-aligned, so the
            # reciprocals run over [32:64]/[96:128]; rows 32:48/96:112 are
            # junk reciprocals of V-data that nothing reads.
            zr = zpool.tile([128, 512], F32, tag="zr")
            nc.vector.reciprocal(zr[32:64, :], oAB[32:64, :])
            nc.vector.reciprocal(zr[96:128, :], oAB[96:128, :])
            # partition-replicate the 16 Z rows x3 -> 48 rows
            zsA = zpool.tile([48, 512], F32, tag="zsA")
            for r in range(3):
                nc.sync.dma_start(out=zsA[16 * r : 16 * r + 16, :],
                                  in_=zr[48:64, :])
            stA = stpool.tile([48, 512], F32R, tag="stA")
            nc.vector.tensor_mul(stA, oAB[0:48, :], zsA)
            zsB = zpool.tile([48, 512], F32, tag="zsB")
            for r in range(3):
                nc.sync.dma_start(out=zsB[16 * r : 16 * r + 16, :],
                                  in_=zr[112:128, :])
            stB = stpool.tile([48, 512], F32R, tag="stB")
            nc.vector.tensor_mul(stB, oAB[64:112, :], zsB)

            # repack into dense attn^T rows [h*48, h*48+48)
            c = (c5 * 512) // CQ
            col = (c5 * 512) % CQ
            for h, src in ((hA, stA), (hB, stB)):
                r0 = h * DK
                d0, o0 = r0 // 128, r0 % 128
                n0 = min(48, 128 - o0)
                nc.sync.dma_start(
                    out=attn_dense[c][d0][o0 : o0 + n0, col : col + 512],
                    in_=src[0:n0, :],
                )
                if n0 < 48:
                    nc.sync.dma_start(
                        out=attn_dense[c][d0 + 1][0 : 48 - n0, col : col + 512],
                        in_=src[n0:48, :],
                    )

        # tail: whatever remains (last rep's final fc chunk)
        for g in sorted(sched):
            for fn, args in sched.pop(g, ()):
                fn(*args)


_NC_CACHE = None


def _get_nc():
    global _NC_CACHE
    if _NC_CACHE is None:
        _NC_CACHE = build_nc()
    return _NC_CACHE


def kernel(x, W_qkv, W_o, b_o):
    x = np.asarray(x, dtype=np.float32)
    W_qkv = np.ascontiguousarray(np.asarray(W_qkv, dtype=np.float32))
    W_o = np.ascontiguousarray(np.asarray(W_o, dtype=np.float32))
    b_o = np.ascontiguousarray(np.asarray(b_o, dtype=np.float32))
    b, p, n, d = x.shape
    assert (b, p, n, d) == (NCORES, 1, N, D), x.shape

    nc = _get_nc()
    in_maps = [
        {
            "x": np.ascontiguousarray(x[i, 0]),
            "W_qkv": W_qkv,
            "W_o": W_o,
            "b_o": b_o,
        }
        for i in range(NCORES)
    ]
    res = run_bass_kernel_spmd(nc, in_maps, core_ids=list(range(NCORES)))
    outs = np.stack([res.results[i]["out"] for i in range(NCORES)])
    return outs[:, None].astype(np.float32)
